# revision 1
# baseline (speedup 1.0000x reference)
"""ContextWeaver: context[i, j] = relu(sum_{k,d} node[i,k,d] * edge[j,k,d]), diag zeroed.

Strategy (8 NeuronCores, SPMD):
  - Shard node rows 8-way (1024 rows/core); replicate edge^T per core with a
    per-core column rotation of c*1024 so the diagonal block lands at local
    columns [m*128, (m+1)*128) of every 128-row strip -- the instruction
    stream is identical on all cores and diagonal masking is fully static.
  - Contraction dim is 64 (= K*D); pack two independent 64-row matmuls into
    the 128x128 PE array with tile_position row tiling: partitions 0-63
    compute local columns [0, 4096), partitions 64-127 compute [4096, 8192).
  - PSUM -> SBUF relu split between ScalarE (Relu activation) and VectorE
    (tensor_scalar_max); per-strip [128, 8192] staging, 1 MB output DMAs.
  - Host unshards by rotating each slab back and stacking.
"""

import numpy as np

import concourse.bass as bass
import concourse.mybir as mybir
import concourse.tile as tile
from concourse import bacc
from concourse.bass_utils import run_bass_kernel_spmd

N = 8192          # nodes
F = 64            # contraction (K*D = 2*32)
NCORES = 8
SHARD = N // NCORES        # 1024 rows per core
HALF = N // 2              # 4096 local columns per PE row-group
MT = 128                   # output-row strip height
NT = 512                   # matmul moving free dim (one PSUM bank fp32)
DMA_CHUNK = 2048           # output DMA width (1 MiB per dma_start)

F32 = mybir.dt.float32


def build_nc():
    nc = bacc.Bacc("TRN2", target_bir_lowering=False, debug=False)

    node2_d = nc.dram_tensor("node2", [128, SHARD], F32, kind="ExternalInput")
    edge2_d = nc.dram_tensor("edge2", [128, HALF], F32, kind="ExternalInput")
    mask_d = nc.dram_tensor("dmask", [128, MT], F32, kind="ExternalInput")
    out_d = nc.dram_tensor("out", [SHARD, N], F32, kind="ExternalOutput")

    n_strips = SHARD // MT           # 8
    n_chunks = HALF // NT            # 8 matmul pairs per strip

    with tile.TileContext(nc) as tc:
        with (
            tc.tile_pool(name="consts", bufs=1) as consts,
            tc.tile_pool(name="outp", bufs=2) as outp,
            tc.tile_pool(name="psp", bufs=4, space=bass.MemorySpace.PSUM) as psp,
        ):
            node_sb = consts.tile([128, SHARD], F32)
            mask_sb = consts.tile([128, MT], F32)
            edge_sb = consts.tile([128, HALF], F32)

            nc.sync.dma_start(out=node_sb[:], in_=node2_d[:, :])
            nc.sync.dma_start(out=mask_sb[:], in_=mask_d[:, :])
            # chunked so the first matmuls only wait on their own column range
            for j in range(n_chunks):
                nc.sync.dma_start(
                    out=edge_sb[:, j * NT:(j + 1) * NT],
                    in_=edge2_d[:, j * NT:(j + 1) * NT],
                )

            for m in range(n_strips):
                strip = outp.tile([128, N], F32)
                lhs_lo = node_sb[0:64, m * MT:(m + 1) * MT]
                lhs_hi = node_sb[64:128, m * MT:(m + 1) * MT]
                for n in range(n_chunks):
                    ps_a = psp.tile([128, NT], F32)
                    ps_b = psp.tile([128, NT], F32)
                    nc.tensor.matmul(
                        ps_a[:],
                        lhs_lo,
                        edge_sb[0:64, n * NT:(n + 1) * NT],
                        start=True, stop=True,
                        tile_position=(0, 0),
                    )
                    nc.tensor.matmul(
                        ps_b[:],
                        lhs_hi,
                        edge_sb[64:128, n * NT:(n + 1) * NT],
                        start=True, stop=True,
                        tile_position=(64, 0),
                    )
                    nc.scalar.activation(
                        strip[:, n * NT:(n + 1) * NT], ps_a[:],
                        mybir.ActivationFunctionType.Relu,
                    )
                    nc.vector.tensor_scalar_max(
                        strip[:, HALF + n * NT:HALF + (n + 1) * NT], ps_b[:], 0.0,
                    )
                # zero the diagonal block (always local cols [m*MT, (m+1)*MT))
                nc.vector.tensor_mul(
                    strip[:, m * MT:(m + 1) * MT],
                    strip[:, m * MT:(m + 1) * MT],
                    mask_sb[:],
                )
                for q in range(N // DMA_CHUNK):
                    nc.sync.dma_start(
                        out=out_d[m * MT:(m + 1) * MT, q * DMA_CHUNK:(q + 1) * DMA_CHUNK],
                        in_=strip[:, q * DMA_CHUNK:(q + 1) * DMA_CHUNK],
                    )

    nc.compile()
    return nc


_NC = None


def _get_nc():
    global _NC
    if _NC is None:
        _NC = build_nc()
    return _NC


def make_in_maps(node_features: np.ndarray, edge_features: np.ndarray):
    node = np.ascontiguousarray(node_features, dtype=np.float32).reshape(N, F)
    edge = np.ascontiguousarray(edge_features, dtype=np.float32).reshape(N, F)
    edge_t = np.ascontiguousarray(edge.T)                      # [64, 8192]
    mask = np.ones((128, MT), np.float32)
    np.fill_diagonal(mask, 0.0)

    in_maps = []
    for c in range(NCORES):
        node_t = node[c * SHARD:(c + 1) * SHARD].T             # [64, 1024]
        node2 = np.ascontiguousarray(np.concatenate([node_t, node_t], axis=0))
        et = np.roll(edge_t, -c * SHARD, axis=1)               # local col j' = global (j'+c*1024)%N
        edge2 = np.ascontiguousarray(np.concatenate([et[:, :HALF], et[:, HALF:]], axis=0))
        in_maps.append({"node2": node2, "edge2": edge2, "dmask": mask})
    return in_maps


def kernel(node_features: np.ndarray, edge_features: np.ndarray) -> np.ndarray:
    nc = _get_nc()
    in_maps = make_in_maps(node_features, edge_features)
    res = run_bass_kernel_spmd(nc, in_maps, core_ids=list(range(NCORES)))
    out = np.empty((N, N), np.float32)
    for c in range(NCORES):
        out[c * SHARD:(c + 1) * SHARD] = np.roll(res.results[c]["out"], c * SHARD, axis=1)
    return out


# revision 19
# speedup vs baseline: 121471.4940x; 121471.4940x over previous
"""ContextWeaver: context[i, j] = relu(sum_{k,d} node[i,k,d] * edge[j,k,d]), diag zeroed.

Strategy (8 NeuronCores, SPMD):
  - Shard node rows 8-way (1024 rows/core); replicate edge^T per core with a
    per-core column rotation of c*1024 so the diagonal block lands at local
    columns [m*128, (m+1)*128) of every 128-row strip -- the instruction
    stream is identical on all cores and diagonal masking is fully static.
  - Contraction dim is 64 (= K*D); pack two independent 64-row matmuls into
    the 128x128 PE array with tile_position row tiling: partitions 0-63
    compute local columns [0, 4096), partitions 64-127 compute [4096, 8192).
  - PSUM -> SBUF relu split between ScalarE (Relu activation) and VectorE
    (tensor_scalar_max); per-strip [128, 8192] staging, 1 MB output DMAs.
  - Host unshards by rotating each slab back and stacking.
"""

import numpy as np

import concourse.bass as bass
import concourse.mybir as mybir
import concourse.tile as tile
from concourse import bacc
from concourse.bass_utils import run_bass_kernel_spmd

N = 8192          # nodes
F = 64            # contraction (K*D = 2*32)
NCORES = 8
SHARD = N // NCORES        # 1024 rows per core
HALF = N // 2              # 4096 local columns per PE row-group
MT = 128                   # output-row strip height
NT = 512                   # matmul moving free dim (one PSUM bank fp32)
import os as _os
DMA_CHUNK = int(_os.environ.get("KL_DMA_CHUNK", "2048"))   # output DMA width
DUAL_RING = _os.environ.get("KL_DUAL_RING", "1") == "1"    # alternate sync/scalar HWDGE rings
DVE_DUP = _os.environ.get("KL_DVE_DUP", "1") == "1"        # duplicate nodeT on-chip via DVE

F32 = mybir.dt.float32


def build_nc():
    nc = bacc.Bacc("TRN2", target_bir_lowering=False, debug=False)

    node2_d = nc.dram_tensor(
        "node2", [64 if DVE_DUP else 128, SHARD], F32, kind="ExternalInput"
    )
    edge2_d = nc.dram_tensor("edge2", [128, HALF], F32, kind="ExternalInput")
    mask_d = nc.dram_tensor("dmask", [128, MT], F32, kind="ExternalInput")
    out_d = nc.dram_tensor("out", [SHARD, N], F32, kind="ExternalOutput")

    n_strips = SHARD // MT           # 8
    n_chunks = HALF // NT            # 8 matmul pairs per strip

    with tile.TileContext(nc) as tc:
        with (
            tc.tile_pool(name="consts", bufs=1) as consts,
            tc.tile_pool(name="outp", bufs=3) as outp,
            tc.tile_pool(name="psp", bufs=4, space=bass.MemorySpace.PSUM) as psp,
        ):
            node_sb = consts.tile([128, SHARD], F32)
            mask_sb = consts.tile([128, MT], F32)
            edge_sb = consts.tile([128, HALF], F32)

            # ordered so the bytes gating the first matmul pair land first:
            # edge chunk 0, node strip 0, mask, then the rest interleaved
            nodedst = node_sb[0:64, :] if DVE_DUP else node_sb[:]
            nc.sync.dma_start(out=edge_sb[:, 0:NT], in_=edge2_d[:, 0:NT])
            nc.sync.dma_start(out=nodedst[:, 0:MT], in_=node2_d[:, 0:MT])
            nc.sync.dma_start(out=mask_sb[:], in_=mask_d[:, :])
            for j in range(1, n_chunks):
                nc.sync.dma_start(
                    out=edge_sb[:, j * NT:(j + 1) * NT],
                    in_=edge2_d[:, j * NT:(j + 1) * NT],
                )
            nc.sync.dma_start(out=nodedst[:, MT:], in_=node2_d[:, MT:])
            if DVE_DUP:
                nc.vector.tensor_copy(node_sb[64:128, 0:MT], node_sb[0:64, 0:MT])
                nc.vector.tensor_copy(node_sb[64:128, MT:], node_sb[0:64, MT:])

            for m in range(n_strips):
                strip = outp.tile([128, N], F32)
                lhs_lo = node_sb[0:64, m * MT:(m + 1) * MT]
                lhs_hi = node_sb[64:128, m * MT:(m + 1) * MT]
                for n in range(n_chunks):
                    ps_a = psp.tile([128, NT], F32)
                    ps_b = psp.tile([128, NT], F32)
                    nc.tensor.matmul(
                        ps_a[:],
                        lhs_lo,
                        edge_sb[0:64, n * NT:(n + 1) * NT],
                        start=True, stop=True,
                        tile_position=(0, 0),
                    )
                    nc.tensor.matmul(
                        ps_b[:],
                        lhs_hi,
                        edge_sb[64:128, n * NT:(n + 1) * NT],
                        start=True, stop=True,
                        tile_position=(64, 0),
                    )
                    nc.scalar.activation(
                        strip[:, n * NT:(n + 1) * NT], ps_a[:],
                        mybir.ActivationFunctionType.Relu,
                    )
                    nc.vector.tensor_scalar_max(
                        strip[:, HALF + n * NT:HALF + (n + 1) * NT], ps_b[:], 0.0,
                    )
                # zero the diagonal block (always local cols [m*MT, (m+1)*MT))
                nc.vector.tensor_mul(
                    strip[:, m * MT:(m + 1) * MT],
                    strip[:, m * MT:(m + 1) * MT],
                    mask_sb[:],
                )
                for q in range(N // DMA_CHUNK):
                    eng = nc.scalar if (DUAL_RING and q % 2 == 1) else nc.sync
                    eng.dma_start(
                        out=out_d[m * MT:(m + 1) * MT, q * DMA_CHUNK:(q + 1) * DMA_CHUNK],
                        in_=strip[:, q * DMA_CHUNK:(q + 1) * DMA_CHUNK],
                    )

    nc.compile()
    return nc


_NC = None


def _get_nc():
    global _NC
    if _NC is None:
        _NC = build_nc()
    return _NC


def make_in_maps(node_features: np.ndarray, edge_features: np.ndarray):
    node = np.ascontiguousarray(node_features, dtype=np.float32).reshape(N, F)
    edge = np.ascontiguousarray(edge_features, dtype=np.float32).reshape(N, F)
    edge_t = np.ascontiguousarray(edge.T)                      # [64, 8192]
    mask = np.ones((128, MT), np.float32)
    np.fill_diagonal(mask, 0.0)

    in_maps = []
    for c in range(NCORES):
        node_t = node[c * SHARD:(c + 1) * SHARD].T             # [64, 1024]
        if DVE_DUP:
            node2 = np.ascontiguousarray(node_t)
        else:
            node2 = np.ascontiguousarray(np.concatenate([node_t, node_t], axis=0))
        et = np.roll(edge_t, -c * SHARD, axis=1)               # local col j' = global (j'+c*1024)%N
        edge2 = np.ascontiguousarray(np.concatenate([et[:, :HALF], et[:, HALF:]], axis=0))
        in_maps.append({"node2": node2, "edge2": edge2, "dmask": mask})
    return in_maps


def kernel(node_features: np.ndarray, edge_features: np.ndarray) -> np.ndarray:
    nc = _get_nc()
    in_maps = make_in_maps(node_features, edge_features)
    res = run_bass_kernel_spmd(nc, in_maps, core_ids=list(range(NCORES)))
    out = np.empty((N, N), np.float32)
    for c in range(NCORES):
        out[c * SHARD:(c + 1) * SHARD] = np.roll(res.results[c]["out"], c * SHARD, axis=1)
    return out
